# revision 19
# baseline (speedup 1.0000x reference)
"""Trainium2 Bass kernel for nn_Head_44203803411019.

Single attention head, B=16 T=2048 C=768 HS=64, fp32, with the source
quirks: scores scaled by 1/sqrt(C) (not head size) and softmax over the
QUERY axis (axis=1), i.e. a column softmax of the causal-masked score
matrix.

Math: with P = exp(S_masked) (no max-subtraction needed: |S| is tiny)
and c[k] = sum_q P[q,k], the output is
    out = W @ V = P @ (V / c[:, None])
so the big TxT matrix is never normalized; V rows are scaled instead.

Dataflow per example (bf16 compute, fp32 accumulation):
  x --DMA--> x fp32 --DVE cast--> x bf16 --xbar-transpose-DMA--> xT [c,q]
  QKV nat [q, 192] = xT_tile.T @ (Wq|Wk|Wv)       (PE, psum fp32)
  (Q|K) nat --xbar-transpose-DMA--> qkt [64, {Q^T,K^T}, T]   bf16
  S^T[k,q] = KT_kb.T @ QT   per 128-row k-block (valid q >= k only)
  P^T = exp(scl*S^T + diagmask)  bf16, accum_out -> column sums
  Vn = V * (1/c)                                   (DVE, bf16)
  out^T[h,q] = sum_kb Vn[kb].T @ P^T[kb]           (PE, psum fp32)
  out^T --PE fp32 transpose--> out natural --DMA--> y fp32

Sharding: data-parallel over batch, 2 examples per core, weights
replicated.  Inputs are the FULL tensors; sharding happens in kernel().
The body sits in a tc.For_i(0, reps) hardware loop so one NEFF can
repeat the computation for timing (NEFF size is rep-count independent).
"""

import math
import os

import numpy as np

import concourse.bass as bass
import concourse.mybir as mybir
from concourse.masks import make_identity
from concourse.tile import TileContext
from concourse.vector_clock import ScopedClock, VectorClock

try:
    from concourse.tile_sem_assignment import N_PROCS
except ImportError:  # pragma: no cover
    N_PROCS = 27

B, T, C, HS = 16, 2048, 768, 64
NCORES = 8
BPC = B // NCORES          # examples per core
P = 128                    # SBUF partitions
NT = T // P                # 16 t-blocks
NCB = C // P               # 6 c-blocks
QG = 512                   # q-chunk width (PSUM bank)
NQG = T // QG              # 4 q-chunks
SCL = 1.0 / math.sqrt(C)
MASK_NEG = -1.0e5

F32 = mybir.dt.float32
F32R = mybir.dt.float32r
BF16 = mybir.dt.bfloat16

# column offset of k-block kb inside the packed triangular PT buffer
PTOFF = [0] * (NT + 1)
for _kb in range(NT):
    PTOFF[_kb + 1] = PTOFF[_kb] + (T - _kb * P)
PTW = PTOFF[NT]            # 17408 columns total


class _SplitDrainTileContext(TileContext):
    """TileContext whose tail drain splits its sem waits across several
    drain instructions: this neuronxcc build caps sync-wait commands per
    CTRL instruction and rejects the stock single drain-with-N-waits."""

    def _drain_and_barrier(self, tick_clock, wait_clock):
        gc = tick_clock.global_clock
        for p in range(N_PROCS):
            if gc[p] <= 0:
                continue
            partial = VectorClock(
                [gc[q] if q == p else 0 for q in range(N_PROCS)]
            )
            drain_inst = self.nc.sync.drain()
            wait_clock.add_sem_waits(
                drain_inst.ins, ScopedClock({None: partial})
            )
        self.nc.all_engine_barrier()
        popped = self.nc._tile_sem_poison_stack.pop()
        assert popped is self._sem_poison
        self.nc.clear_and_free_semaphores(list(self.sems.allocated().values()))
        self.nc.all_engine_barrier()


def _split_sync_waits(nc, maxw=1):
    """This neuronxcc build rejects >1 sync-wait command on several
    instruction structs (CTRL drains, matmul LDW).  Move excess waits onto
    dedicated same-engine NOPs placed right before the instruction."""
    k = 0
    for f in nc.m.functions:
        for bb in f.blocks:
            new = []
            for inst in bb.instructions:
                si = inst.sync_info
                waits = list(si.on_wait) if si is not None and si.on_wait else []
                if len(waits) > maxw:
                    extra, keep = waits[:-maxw], waits[-maxw:]
                    for i in range(0, len(extra), maxw):
                        k += 1
                        new.append(
                            mybir.InstNoOp(
                                name=f"{inst.name}_sw{k}",
                                engine=inst.engine,
                                bass_nofuse=True,
                                sync_info=mybir.SyncInfo(
                                    on_wait=extra[i:i + maxw], on_update=[]
                                ),
                            )
                        )
                    si.on_wait = keep
                new.append(inst)
            bb.instructions[:] = new


def _build_nc(reps: int = 1) -> bass.Bass:
    stages = os.environ.get("KSTAGES", "ABCD")
    nc = bass.Bass()
    x_in = nc.declare_dram_parameter("x", [BPC, T, C], F32, isOutput=False)
    wk_in = nc.declare_dram_parameter("wk", [C, HS], F32, isOutput=False)
    wq_in = nc.declare_dram_parameter("wq", [C, HS], F32, isOutput=False)
    wv_in = nc.declare_dram_parameter("wv", [C, HS], F32, isOutput=False)
    y_out = nc.declare_dram_parameter("out", [BPC, T, HS], F32, isOutput=True)

    with _SplitDrainTileContext(nc) as tc:
        with (
            tc.tile_pool(name="singles", bufs=1) as singles,
            tc.tile_pool(name="xf", bufs=2) as p_xf,
            tc.tile_pool(name="xb", bufs=2) as p_xb,
            tc.tile_pool(name="xt", bufs=2) as p_xt,
            tc.tile_pool(name="qkt", bufs=2) as p_qkt,
            tc.tile_pool(name="vv", bufs=2) as p_vv,
            tc.tile_pool(name="pt", bufs=2) as p_pt,
            tc.tile_pool(name="small", bufs=2) as p_small,
            tc.tile_pool(name="stats", bufs=2) as p_stats,
            tc.tile_pool(name="psA", bufs=2, space="PSUM") as p_psA,
            tc.tile_pool(name="psS", bufs=2, space="PSUM") as p_psS,
            tc.tile_pool(name="psO", bufs=1, space="PSUM") as p_psO,
            tc.tile_pool(name="psT", bufs=1, space="PSUM") as p_psT,
        ):
            ident = singles.tile([P, P], F32)
            make_identity(nc, ident)
            identr = singles.tile([P, P], F32R, tag="identr")
            nc.vector.tensor_copy(identr, ident)

            # diag mask: 0 where q-col >= k-row, MASK_NEG below the diagonal
            dmask = singles.tile([P, P], F32)
            nc.gpsimd.memset(dmask, 0.0)
            nc.gpsimd.affine_select(
                out=dmask,
                in_=dmask,
                compare_op=mybir.AluOpType.is_ge,
                fill=MASK_NEG,
                base=0,
                pattern=[[1, P]],
                channel_multiplier=-1,
            )

            # weights, cast to bf16, concatenated on free axis: [c, Q|K|V]
            w3 = singles.tile([P, NCB, 3 * HS], BF16, tag="w3")
            for i, src in enumerate((wq_in, wk_in, wv_in)):
                wf = singles.tile([P, NCB, HS], F32, tag=f"wf{i}")
                nc.scalar.dma_start(
                    out=wf, in_=src.rearrange("(cb p) h -> p cb h", p=P)
                )
                nc.vector.tensor_copy(w3[:, :, i * HS:(i + 1) * HS], wf)

            with tc.For_i(0, reps, name="reps"):
                qkts, kqts, vnats = {}, {}, {}
                for b in range(BPC):
                    # ------------- stage A: load, transpose, project ----
                    xt = p_xt.tile([P, NT, NCB, P], BF16, tag="xt")
                    # qkt: partitions 0-63 = Q^T, 64-127 = K^T
                    # kqt: partitions 0-63 = K^T, 64-127 = Q^T
                    qkt = p_qkt.tile([P, NT, P], BF16, tag="qkt")
                    qkts[b] = qkt
                    kqt = p_qkt.tile([P, NT, P], BF16, tag="kqt")
                    kqts[b] = kqt
                    vnat = p_vv.tile([P, NT, HS], F32, tag="vnat")
                    vnats[b] = vnat

                    if "A" not in stages:
                        continue
                    for g in range(NQG):
                        xf = p_xf.tile([P, 4, C], F32, tag="xf")
                        nc.scalar.dma_start(
                            out=xf,
                            in_=x_in[b, g * QG:(g + 1) * QG, :].rearrange(
                                "(tt p) c -> p tt c", p=P
                            ),
                        )
                        xb = p_xb.tile([P, 4, C], BF16, tag="xb")
                        nc.vector.tensor_copy(xb, xf)
                        nc.sync.dma_start(
                            out=xt[:, g * 4:(g + 1) * 4, :, :],
                            in_=xb,
                            transpose=True,
                        )
                    for g in range(NQG):
                        qk_big = p_small.tile([P, 4, 2 * HS], BF16, tag="qks")
                        kq_big = p_small.tile([P, 4, 2 * HS], BF16, tag="kqs")
                        for tt in range(4):
                            qt = g * 4 + tt
                            ps_qkv = p_psA.tile([P, 3 * HS], F32, tag="qkv")
                            for cb in range(NCB):
                                nc.tensor.matmul(
                                    ps_qkv,
                                    xt[:, qt, cb, :],
                                    w3[:, cb, :],
                                    start=(cb == 0),
                                    stop=(cb == NCB - 1),
                                )
                            nc.vector.tensor_copy(
                                qk_big[:, tt, :], ps_qkv[:, 0:2 * HS]
                            )
                            nc.vector.tensor_copy(
                                kq_big[:, tt, 0:HS], ps_qkv[:, HS:2 * HS]
                            )
                            nc.vector.tensor_copy(
                                kq_big[:, tt, HS:2 * HS], ps_qkv[:, 0:HS]
                            )
                            nc.vector.tensor_copy(
                                vnat[:, qt, :], ps_qkv[:, 2 * HS:3 * HS]
                            )
                        nc.sync.dma_start(
                            out=qkt[:, g * 4:(g + 1) * 4, :],
                            in_=qk_big,
                            transpose=True,
                        )
                        nc.sync.dma_start(
                            out=kqt[:, g * 4:(g + 1) * 4, :],
                            in_=kq_big,
                            transpose=True,
                        )

                for b in range(BPC):
                    qkt = qkts[b]
                    kqt = kqts[b]
                    vnat = vnats[b]
                    # ------------- stage B: scores + exp + col sums -----
                    if "B" not in stages:
                        continue
                    pt = p_pt.tile([P, PTW], BF16, tag="pt")
                    ctile = p_stats.tile([P, NT, NQG], F32, tag="ctile")
                    nc.vector.memset(ctile, 0.0)

                    # k-blocks in pairs: even kb on PE rows 0-63, odd kb
                    # on rows 64-127 (tile_position) -> concurrent matmuls
                    for j in range(NT // 2):
                        for qg in range(j // 2, NQG):
                            sub = []
                            for i, (kb, lt, rt, tp) in enumerate((
                                (2 * j, kqt, qkt, None),
                                (2 * j + 1, qkt, kqt, (HS, 0)),
                            )):
                                q0 = kb * P
                                qs = max(q0, qg * QG)
                                w = (qg + 1) * QG - qs
                                if w <= 0:
                                    continue
                                lo = HS * i
                                ps_s = p_psS.tile([P, QG], F32, tag=f"s{i}")
                                nc.tensor.matmul(
                                    ps_s[:, :w],
                                    lt[lo:lo + HS, q0 // P, :],
                                    rt[lo:lo + HS, qs // P:(qs + w) // P, :],
                                    start=True,
                                    stop=True,
                                    tile_position=tp,
                                )
                                sub.append((kb, q0, qs, w, ps_s))
                            for kb, q0, qs, w, ps_s in sub:
                                if qs == q0:
                                    nc.vector.tensor_add(
                                        ps_s[:, 0:P], ps_s[:, 0:P], dmask
                                    )
                                nc.scalar.activation(
                                    out=pt[:, PTOFF[kb] + qs - q0:
                                           PTOFF[kb] + qs - q0 + w],
                                    in_=ps_s[:, :w],
                                    func=mybir.ActivationFunctionType.Exp,
                                    scale=SCL,
                                    accum_out=ctile[
                                        :, kb,
                                        qg - q0 // QG: qg - q0 // QG + 1
                                    ],
                                )

                    # ------------- stage C: Vn = V / colsum -------------
                    if "C" not in stages:
                        continue
                    cagg = p_stats.tile([P, NT], F32, tag="cagg")
                    nc.vector.reduce_sum(
                        cagg, ctile, axis=mybir.AxisListType.X
                    )
                    cinv = p_stats.tile([P, NT], F32, tag="cinv")
                    nc.vector.reciprocal(cinv, cagg)
                    vn = p_vv.tile([P, NT, HS], BF16, tag="vn")
                    for tb in range(NT):
                        nc.vector.tensor_scalar_mul(
                            vn[:, tb, :],
                            vnat[:, tb, :],
                            cinv[:, tb:tb + 1],
                        )

                    # ------------- stage D: out = P @ V' ----------------
                    if "D" not in stages:
                        continue
                    for qg in range(NQG):
                        ps_o = p_psO.tile([HS, QG], F32, tag="o")
                        nkb = 4 * (qg + 1)
                        for kb in range(nkb):
                            q0 = kb * P
                            qs = max(q0, qg * QG)
                            w = (qg + 1) * QG - qs
                            nc.tensor.matmul(
                                ps_o[:, qs - qg * QG: qs - qg * QG + w],
                                vn[:, kb, :],
                                pt[:, PTOFF[kb] + qs - q0:
                                   PTOFF[kb] + qs - q0 + w],
                                start=(kb == 0),
                                stop=(kb == nkb - 1),
                            )
                        ot = p_small.tile([HS, QG], F32R, tag="ot")
                        nc.vector.tensor_copy(ot, ps_o.bitcast(F32R))
                        ps_on = p_psT.tile([P, 4 * HS], F32R, tag="tr")
                        for tt in range(4):
                            nc.tensor.transpose(
                                ps_on[:, tt * HS:(tt + 1) * HS],
                                ot[:, tt * P:(tt + 1) * P],
                                identr[0:HS, 0:HS],
                            )
                        on = p_small.tile([P, 4, HS], F32, tag="on")
                        nc.vector.tensor_copy(on, ps_on.bitcast(F32))
                        nc.scalar.dma_start(
                            out=y_out[b, qg * QG:(qg + 1) * QG, :].rearrange(
                                "(tt p) h -> p tt h", p=P
                            ),
                            in_=on,
                        )
    _split_sync_waits(nc)
    return nc


_RUNNER_CACHE = {}


def _make_runner(reps: int):
    """Build nc once, jit once; repeat calls only pay transfer + exec."""
    import jax
    import numpy as _np
    from jax.sharding import Mesh, PartitionSpec
    from jax.experimental.shard_map import shard_map

    from concourse import bass2jax

    nc = _build_nc(reps)
    bass2jax.install_neuronx_cc_hook()

    part_name = (
        nc.partition_id_tensor.name if nc.partition_id_tensor else None
    )
    in_names = []
    out_names = []
    out_avals = []
    zero_shapes = []
    for alloc in nc.m.functions[0].allocations:
        if not isinstance(alloc, mybir.MemoryLocationSet):
            continue
        name = alloc.memorylocations[0].name
        if alloc.kind == "ExternalInput":
            if name != part_name:
                in_names.append(name)
        elif alloc.kind == "ExternalOutput":
            out_names.append(name)
            shape = tuple(alloc.tensor_shape)
            dtype = mybir.dt.np(alloc.dtype)
            out_avals.append(jax.core.ShapedArray(shape, dtype))
            zero_shapes.append((shape, dtype))
    n_params = len(in_names)
    all_names = in_names + out_names
    if part_name is not None:
        all_names = all_names + [part_name]

    def _body(*args):
        operands = list(args)
        if part_name is not None:
            operands.append(bass2jax.partition_id_tensor())
        outs = bass2jax._bass_exec_p.bind(
            *operands,
            out_avals=tuple(out_avals),
            in_names=tuple(all_names),
            out_names=tuple(out_names),
            lowering_input_output_aliases=(),
            sim_require_finite=True,
            sim_require_nnan=True,
            nc=nc,
        )
        return tuple(outs)

    devices = jax.devices()[:NCORES]
    mesh = Mesh(_np.asarray(devices), ("core",))
    in_specs = (PartitionSpec("core"),) * (n_params + len(out_names))
    out_specs = (PartitionSpec("core"),) * len(out_names)
    donate = tuple(range(n_params, n_params + len(out_names)))
    fn = jax.jit(
        shard_map(
            _body, mesh=mesh, in_specs=in_specs, out_specs=out_specs,
            check_rep=False,
        ),
        donate_argnums=donate,
        keep_unused=True,
    )

    def run(in_maps):
        concat = [
            np.concatenate([m[name] for m in in_maps], axis=0)
            for name in in_names
        ]
        zeros = [
            np.zeros((NCORES * s[0], *s[1:]), d) for s, d in zero_shapes
        ]
        outs = fn(*concat, *zeros)
        return np.asarray(outs[0])

    return run


def kernel(x, Wk, Wq, Wv, _reps=1):
    """Full-input entry point: shards over batch across 8 NeuronCores."""
    x = np.ascontiguousarray(np.asarray(x, dtype=np.float32))
    Wk = np.ascontiguousarray(np.asarray(Wk, dtype=np.float32))
    Wq = np.ascontiguousarray(np.asarray(Wq, dtype=np.float32))
    Wv = np.ascontiguousarray(np.asarray(Wv, dtype=np.float32))
    assert x.shape == (B, T, C), x.shape

    key = (_reps, os.environ.get("KSTAGES", "ABCD"))
    if key not in _RUNNER_CACHE:
        _RUNNER_CACHE[key] = _make_runner(_reps)
    run = _RUNNER_CACHE[key]

    in_maps = [
        {
            "x": x[i * BPC:(i + 1) * BPC],
            "wk": Wk,
            "wq": Wq,
            "wv": Wv,
        }
        for i in range(NCORES)
    ]
    full = run(in_maps)
    return full.reshape(B, T, HS)
